# revision 4
# baseline (speedup 1.0000x reference)
"""Trainium2 Bass kernel for nn_Attn_32925219291574.

Math: reference computes softmax_s( v . (W @ [hidden; enc[b,s]] + b) ).
Split W = [Wh | We]. The hidden/bias part v.(Wh@hidden + b) is constant in s,
and softmax is shift-invariant, so the output is exactly
    softmax_s( enc[b,s,:] . u ),   u = v @ We    (We = W[:, H:2H])
`hidden` and `b` never affect the output. The kernel streams the 256 MiB
encoder_outputs tensor once (memory-bound), computing per-row dot products
with a fused DVE multiply+reduce, then does a 2D softmax per batch.

Sharding: data-parallel over batch B=16 -> 2 batches per core, no collectives.
"""

import numpy as np
from contextlib import ExitStack

import concourse.bass as bass
import concourse.bacc as bacc
import concourse.tile as tile
from concourse import mybir
from concourse.masks import make_identity
from concourse.bass_utils import run_bass_kernel_spmd

# Problem shapes (hardcoded per contest contract)
B, S, H = 16, 4096, 1024
NCORES = 8
B_LOC = B // NCORES            # 2 batches per core
ROWS = B_LOC * S               # 8192 rows of enc per core
P = 128
N_TILES = ROWS // P            # 64 tiles of [128, 1024]
TILES_PER_CHUNK = 8            # DMA chunk = [128, 8, 1024] = 4 MiB
N_CHUNKS = N_TILES // TILES_PER_CHUNK   # 8
KC = H // P                    # 8 contraction chunks for u = v @ We
TILES_PER_BATCH = S // P       # 32 score columns per batch

F32 = mybir.dt.float32

# set by test.py to capture a profile; harness leaves these untouched
TRACE = False
LAST_RESULT = None


def _emit(ctx: ExitStack, tc: tile.TileContext, enc_h, we_h, v_h, out_h):
    nc = tc.nc
    enc_ap = enc_h[:, :, :]
    we_ap = we_h[:, :]
    v_ap = v_h[:, :]
    out_ap = out_h[:, :, :]

    singles = ctx.enter_context(tc.tile_pool(name="singles", bufs=1))
    chunks = ctx.enter_context(tc.tile_pool(name="chunks", bufs=3))
    smalls = ctx.enter_context(tc.tile_pool(name="smalls", bufs=2))
    psum_u_pool = ctx.enter_context(tc.tile_pool(name="psum_u", bufs=1, space="PSUM"))
    psum_sm = ctx.enter_context(tc.tile_pool(name="psum_sm", bufs=1, space="PSUM"))

    # ---------------- phase 0: u = v @ We, broadcast to all 128 partitions --
    # We rows (contraction index k) on partitions: we_sb[p, kc, h] = We[kc*128+p, h]
    we_sb = singles.tile([P, KC, H], F32)
    nc.sync.dma_start(out=we_sb, in_=we_ap.rearrange("(kc p) h -> p kc h", p=P))
    # v as columns: v_sb[p, kc] = v[kc*128+p]
    v_sb = singles.tile([P, KC], F32)
    nc.sync.dma_start(out=v_sb, in_=v_ap[0, :].rearrange("(kc p) -> p kc", p=P))

    ones_pp = singles.tile([P, P], F32)
    nc.vector.memset(ones_pp, 1.0)
    # vb[:, kc, :] = v chunk broadcast along free dim -> matmul stationary lhsT
    vb = singles.tile([P, KC, P], F32)
    for kc in range(KC):
        nc.vector.tensor_scalar_mul(
            out=vb[:, kc, :], in0=ones_pp, scalar1=v_sb[:, kc : kc + 1]
        )

    # u_bcast[p, h] = sum_k v[k] * We[k, h] for every partition p
    psum_u = psum_u_pool.tile([P, H], F32)
    for nh in range(2):
        for kc in range(KC):
            nc.tensor.matmul(
                psum_u[:, nh * 512 : (nh + 1) * 512],
                lhsT=vb[:, kc, :],
                rhs=we_sb[:, kc, nh * 512 : (nh + 1) * 512],
                start=(kc == 0),
                stop=(kc == KC - 1),
            )
    u_bcast = singles.tile([P, H], F32)
    nc.vector.tensor_copy(out=u_bcast, in_=psum_u)

    # ---------------- main loop: scores[r] = enc_row[r] . u ----------------
    scores = singles.tile([P, N_TILES], F32)   # col c*8+t, row p -> flat row
    scratch = singles.tile([P, H], F32)        # TTR mandatory full-product dump
    enc_r = enc_ap.flatten_outer_dims().rearrange(
        "(c t p) h -> c p t h", p=P, t=TILES_PER_CHUNK
    )
    for c in range(N_CHUNKS):
        ch = chunks.tile([P, TILES_PER_CHUNK, H], F32)
        nc.sync.dma_start(out=ch, in_=enc_r[c])
        for t in range(TILES_PER_CHUNK):
            col = c * TILES_PER_CHUNK + t
            # fused multiply+row-sum on DVE via standard TensorScalarPtr:
            # out = (in0 * 1.0) * in1, accum_out = sum(out)
            nc.vector.scalar_tensor_tensor(
                out=scratch,
                in0=ch[:, t, :],
                scalar=1.0,
                in1=u_bcast,
                op0=mybir.AluOpType.mult,
                op1=mybir.AluOpType.mult,
                accum_out=scores[:, col : col + 1],
            )

    # ---------------- softmax over each batch's 4096 scores ----------------
    identity = singles.tile([P, P], F32)
    make_identity(nc, identity)
    ones_row = singles.tile([1, P], F32)
    nc.vector.memset(ones_row, 1.0)
    ones_col = singles.tile([P, 1], F32)
    nc.vector.memset(ones_col, 1.0)

    for b in range(B_LOC):
        sb = scores[:, b * TILES_PER_BATCH : (b + 1) * TILES_PER_BATCH]  # [128,32]
        # global max: per-partition max -> transpose -> max -> -M
        m1 = smalls.tile([P, 1], F32)
        nc.vector.tensor_reduce(out=m1, in_=sb, axis=mybir.AxisListType.X,
                                op=mybir.AluOpType.max)
        p_m1T = psum_sm.tile([1, P], F32)
        nc.tensor.transpose(p_m1T, m1, identity)
        negM = smalls.tile([1, 1], F32)
        nc.vector.tensor_reduce(out=negM, in_=p_m1T, axis=mybir.AxisListType.X,
                                op=mybir.AluOpType.max, negate=True)
        # broadcast -M to [128,1] via ones_row.T @ negM
        p_negMb = psum_sm.tile([P, 1], F32)
        nc.tensor.matmul(p_negMb, lhsT=ones_row, rhs=negM, start=True, stop=True)
        negMb = smalls.tile([P, 1], F32)
        nc.vector.tensor_copy(out=negMb, in_=p_negMb)
        # P = exp(scores - M), with free per-partition row sums
        pexp = smalls.tile([P, TILES_PER_BATCH], F32)
        s1 = smalls.tile([P, 1], F32)
        nc.scalar.activation(out=pexp, in_=sb,
                             func=mybir.ActivationFunctionType.Exp,
                             bias=negMb, scale=1.0, accum_out=s1)
        # total sum across partitions: s1.T @ ones_col -> [1,1]
        p_S = psum_sm.tile([1, 1], F32)
        nc.tensor.matmul(p_S, lhsT=s1, rhs=ones_col, start=True, stop=True)
        r_S = smalls.tile([1, 1], F32)
        nc.vector.reciprocal(out=r_S, in_=p_S)
        p_rb = psum_sm.tile([P, 1], F32)
        nc.tensor.matmul(p_rb, lhsT=ones_row, rhs=r_S, start=True, stop=True)
        rb = smalls.tile([P, 1], F32)
        nc.vector.tensor_copy(out=rb, in_=p_rb)
        y = smalls.tile([P, TILES_PER_BATCH], F32)
        nc.vector.tensor_scalar_mul(out=y, in0=pexp, scalar1=rb)
        # transpose [128, 32] -> [32, 128] so the HBM store is contiguous
        p_yt = psum_sm.tile([TILES_PER_BATCH, P], F32)
        nc.tensor.transpose(p_yt, y, identity)
        yt = smalls.tile([TILES_PER_BATCH, P], F32)
        nc.vector.tensor_copy(out=yt, in_=p_yt)
        nc.sync.dma_start(
            out=out_ap[b, 0, :].rearrange("(t p) -> t p", p=P), in_=yt
        )


def build_bass():
    nc = bacc.Bacc("TRN2", target_bir_lowering=False)
    enc_h = nc.dram_tensor("enc", [B_LOC, S, H], F32, kind="ExternalInput")
    we_h = nc.dram_tensor("we", [H, H], F32, kind="ExternalInput")
    v_h = nc.dram_tensor("v", [1, H], F32, kind="ExternalInput")
    out_h = nc.dram_tensor("out", [B_LOC, 1, S], F32, kind="ExternalOutput")
    with ExitStack() as ctx:
        tc = ctx.enter_context(tile.TileContext(nc))
        _emit(ctx, tc, enc_h, we_h, v_h, out_h)
    nc.compile()
    return nc


_NC = None


def _get_nc():
    global _NC
    if _NC is None:
        _NC = build_bass()
    return _NC


def kernel(hidden, encoder_outputs, W, b, v):
    global LAST_RESULT
    nc = _get_nc()
    we = np.ascontiguousarray(np.asarray(W, dtype=np.float32)[:, H:])
    v2 = np.ascontiguousarray(np.asarray(v, dtype=np.float32))
    enc = np.asarray(encoder_outputs, dtype=np.float32)
    in_maps = [
        {
            "enc": np.ascontiguousarray(enc[i * B_LOC : (i + 1) * B_LOC]),
            "we": we,
            "v": v2,
        }
        for i in range(NCORES)
    ]
    res = run_bass_kernel_spmd(nc, in_maps, core_ids=list(range(NCORES)),
                               trace=TRACE)
    LAST_RESULT = res
    return np.concatenate([res.results[i]["out"] for i in range(NCORES)], axis=0)
